# revision 4
# baseline (speedup 1.0000x reference)
"""ATR (twin-gate RNN) Trainium2 kernel.

  p = x @ W1.T + b1                       (batched GEMM over all T)
  h_t = sig(p_t+q_t)*p_t + sig(p_t-q_t)*q_t,  q_t = h_{t-1} @ W2.T + b2

Sharding: data-parallel over batch B=128 -> 16 per core across 8 NeuronCores,
zero cross-core communication. All tensors are kept on-device in a
[d-partition, batch-free] (transposed) layout; host prepares/unprepares.

Matmul operands are fp16 (measured ~5e-4 max rel err vs fp32 reference),
accumulation fp32 in PSUM.
"""

import contextlib

import numpy as np

import concourse.bass as bass
import concourse.mybir as mybir
import concourse.tile as tile
from concourse import bacc
from concourse.bass import ts
from concourse.bass_utils import run_bass_kernel_spmd

B, T, D = 128, 256, 1024
NCORES = 8
BL = B // NCORES          # 16 batches per core
P = 128                   # partitions
DT = D // P               # 8 d-tiles
HDT = DT // 2             # 4 (half)
TW = 32                   # t-window per p-GEMM chunk (TW*BL = 512 cols)
NCH = T // TW             # 8 chunks
F16 = mybir.dt.float16
F32 = mybir.dt.float32

_CACHE = {}


def _emit(nc, xT, w1, w2, b1, b2bc, h0, outT):
    tc = nc._tc
    with contextlib.ExitStack() as ctx:
        singles = ctx.enter_context(tc.tile_pool(name="singles", bufs=1))
        w1_sb = singles.tile([P, DT, D], F16)
        nc.sync.dma_start(out=w1_sb, in_=w1)
        w2_sb = singles.tile([P, DT, D], F16)
        nc.sync.dma_start(out=w2_sb, in_=w2)
        b1_sb = singles.tile([P, DT], F32)
        nc.sync.dma_start(out=b1_sb, in_=b1)
        b2_sb = singles.tile([P, DT, BL], F32)
        nc.sync.dma_start(out=b2_sb, in_=b2bc)
        h0_sb = singles.tile([P, DT, BL], F16)
        nc.sync.dma_start(out=h0_sb, in_=h0)
        p_sb = singles.tile([P, DT, T, BL], F16)

        xT_r = xT.rearrange("(a p) t b -> a p t b", p=P)

        # ---- stage 1: p = x @ W1.T + b1, stored transposed as p_sb[e, t, b]
        with (
            tc.tile_pool(name="xin", bufs=3) as xpool,
            tc.tile_pool(name="pps", bufs=4, space="PSUM") as ppsum,
        ):
            for n in range(NCH):
                xn = xpool.tile([P, DT, TW, BL], F16)
                for dt in range(DT):
                    nc.sync.dma_start(out=xn[:, dt], in_=xT_r[dt, :, ts(n, TW), :])
                for e in range(DT):
                    ps = ppsum.tile([P, TW * BL], F32)
                    for dt in range(DT):
                        nc.tensor.matmul(
                            ps,
                            lhsT=w1_sb[:, dt, ts(e, P)],
                            rhs=xn[:, dt].rearrange("p t b -> p (t b)"),
                            start=(dt == 0),
                            stop=(dt == DT - 1),
                        )
                    nc.scalar.activation(
                        out=p_sb[:, e, ts(n, TW), :].rearrange("p t b -> p (t b)"),
                        in_=ps,
                        func=mybir.ActivationFunctionType.Identity,
                        bias=b1_sb[:, e : e + 1],
                        scale=1.0,
                    )

        # ---- stage 2: sequential recurrence over T
        with contextlib.ExitStack() as rctx:
            qps = rctx.enter_context(tc.tile_pool(name="qps", bufs=2, space="PSUM"))
            hp = rctx.enter_context(tc.tile_pool(name="hp", bufs=3))
            gp = rctx.enter_context(tc.tile_pool(name="gp", bufs=3))

            hA = h0_sb[:, 0:HDT, :]
            hB = h0_sb[:, HDT:DT, :]
            for t in range(T):
                # One PSUM tile (= one bank) per (e-half, d-pass): PSUM
                # accumulation-group `start` clears whole-bank has_written
                # flags, so groups must not interleave within a bank. The
                # two d-pass partials are summed on DVE afterwards.
                qt = [
                    [
                        qps.tile(
                            [P, HDT, BL], F32, tag=f"q{eh}{pi}", name=f"q{eh}{pi}_{t}"
                        )
                        for pi in range(2)
                    ]
                    for eh in range(2)
                ]
                # q accumulation in two passes over d-halves so that next
                # step's pass 0 only needs hA (gates half A) -> pipelining.
                for pi, hsrc in ((0, hA), (1, hB)):
                    for e in range(DT):
                        q_h = qt[e // HDT][pi]
                        for dl in range(HDT):
                            nc.tensor.matmul(
                                q_h[:, e % HDT, :],
                                lhsT=w2_sb[:, pi * HDT + dl, ts(e, P)],
                                rhs=hsrc[:, dl, :],
                                start=(dl == 0),
                                stop=(dl == HDT - 1),
                            )
                newh = []
                for hi in range(2):
                    pt = p_sb[:, hi * HDT : (hi + 1) * HDT, t, :]
                    qb0 = gp.tile([P, HDT, BL], F32, tag=f"qb0{hi}")
                    nc.vector.tensor_add(
                        qb0, qt[hi][0], b2_sb[:, hi * HDT : (hi + 1) * HDT, :]
                    )
                    qb = gp.tile([P, HDT, BL], F32, tag=f"qb{hi}")
                    nc.vector.tensor_add(qb, qb0, qt[hi][1])
                    sd = gp.tile([P, 2, HDT, BL], F16, tag=f"sd{hi}")
                    nc.vector.tensor_add(sd[:, 0], pt, qb)
                    nc.vector.tensor_sub(sd[:, 1], pt, qb)
                    ifg = gp.tile([P, 2, HDT, BL], F16, tag=f"ifg{hi}")
                    nc.scalar.activation(
                        out=ifg.rearrange("p s d b -> p (s d b)"),
                        in_=sd.rearrange("p s d b -> p (s d b)"),
                        func=mybir.ActivationFunctionType.Sigmoid,
                    )
                    t1 = gp.tile([P, HDT, BL], F16, tag=f"t1{hi}")
                    nc.vector.tensor_mul(t1, ifg[:, 0], pt)
                    t2 = gp.tile([P, HDT, BL], F16, tag=f"t2{hi}")
                    nc.vector.tensor_mul(t2, ifg[:, 1], qb)
                    hn = hp.tile([P, HDT, BL], F16, tag=f"h{hi}")
                    nc.vector.tensor_add(hn, t1, t2)
                    nc.sync.dma_start(
                        out=outT[t][:, hi * HDT : (hi + 1) * HDT, :], in_=hn
                    )
                    newh.append(hn)
                hA, hB = newh


def build():
    if "nc" in _CACHE:
        return _CACHE["nc"]
    nc = bacc.Bacc("TRN2", target_bir_lowering=False, debug=False, num_devices=NCORES)
    xT = nc.dram_tensor("xT", [D, T, BL], F16, kind="ExternalInput").ap()
    w1 = nc.dram_tensor("w1", [P, DT, D], F16, kind="ExternalInput").ap()
    w2 = nc.dram_tensor("w2", [P, DT, D], F16, kind="ExternalInput").ap()
    b1 = nc.dram_tensor("b1", [P, DT], F32, kind="ExternalInput").ap()
    b2bc = nc.dram_tensor("b2bc", [P, DT, BL], F32, kind="ExternalInput").ap()
    h0 = nc.dram_tensor("h0", [P, DT, BL], F16, kind="ExternalInput").ap()
    outT = nc.dram_tensor("outT", [T, P, DT, BL], F16, kind="ExternalOutput").ap()
    with tile.TileContext(nc) as tc:
        nc._tc = tc
        _emit(nc, xT, w1, w2, b1, b2bc, h0, outT)
    nc.compile()
    _CACHE["nc"] = nc
    return nc


def make_in_maps(x, W1, b1, W2, b2, init_hx):
    x = np.asarray(x, dtype=np.float32)
    W1 = np.asarray(W1, dtype=np.float32)
    b1 = np.asarray(b1, dtype=np.float32)
    W2 = np.asarray(W2, dtype=np.float32)
    b2 = np.asarray(b2, dtype=np.float32)
    init_hx = np.asarray(init_hx, dtype=np.float32)

    w1s = np.ascontiguousarray(
        W1.T.reshape(DT, P, D).transpose(1, 0, 2)
    ).astype(np.float16)  # [din, dtile, e] = W1[e, d]
    w2s = np.ascontiguousarray(
        W2.T.reshape(DT, P, D).transpose(1, 0, 2)
    ).astype(np.float16)
    b1s = np.ascontiguousarray(b1.reshape(DT, P).T)  # [e_in, e_tile]
    b2bc = np.ascontiguousarray(
        np.broadcast_to(b2.reshape(DT, P).T[:, :, None], (P, DT, BL))
    )
    h0 = np.ascontiguousarray(
        np.broadcast_to(init_hx.reshape(DT, P).T[:, :, None], (P, DT, BL))
    ).astype(np.float16)

    in_maps = []
    for c in range(NCORES):
        xc = x[c * BL : (c + 1) * BL]  # [BL, T, D]
        xTc = np.ascontiguousarray(xc.transpose(2, 1, 0)).astype(np.float16)
        in_maps.append(
            {"xT": xTc, "w1": w1s, "w2": w2s, "b1": b1s, "b2bc": b2bc, "h0": h0}
        )
    return in_maps


def assemble(results):
    out = np.empty((B, T, D), dtype=np.float32)
    for c in range(NCORES):
        oT = results[c]["outT"]  # [T, P, DT, BL] f16
        out[c * BL : (c + 1) * BL] = (
            oT.transpose(3, 0, 2, 1).reshape(BL, T, D).astype(np.float32)
        )
    return out


def run(inputs, trace=False):
    nc = build()
    in_maps = make_in_maps(**inputs)
    res = run_bass_kernel_spmd(nc, in_maps, list(range(NCORES)), trace=trace)
    return assemble(res.results), res


def kernel(x, W1, b1, W2, b2, init_hx):
    out, _ = run(dict(x=x, W1=W1, b1=b1, W2=W2, b2=b2, init_hx=init_hx))
    return out
